# revision 37
# baseline (speedup 1.0000x reference)
"""Trainium2 Bass kernel for the DiagonalSSMBlock problem.

Math (per batch, sharded one batch per core over 8 cores):
    a = -exp(log_neg_real) + i*imag ; a_bar = exp(a) = r * e^{i theta}
    b_bar = ((a_bar-1)/a)[:,None] * B
    Bu_t = b_bar @ u_t                         (complex, state dim 64)
    h_t = a_bar * h_{t-1} + Bu_t               (diagonal complex scan over L)
    y_t = Re(C @ h_t) + D*u_t ; out = LN(u + y) * gamma + beta

The kernel is DMA-bound, so HBM traffic is minimized (gate is rel-err
< 2e-2; this pipeline sits at ~5e-3):
  * u ships twice: fp8(e4m3) transposed [d, l] for the Bu matmul (fp8
    feeds the matmul only; 3.5% quantization there decays to ~4e-3 in
    the LN'd output) and bf16 natural [l, d] for the residual
    (pre-scaled by (1+D) on host); both pre-tiled so every per-tile DMA
    is contiguous per partition.  Output is bf16, un-permuted on host.
  * Bu matmul runs fp8 DoubleRow (2 k-subtiles per pass, 2x PE rate);
    b_bar ships fp8 scaled by 64 and the 1/64 compensation is folded
    into the half-mix identity stack.
  * The complex scan is rotated into a per-lane REAL damped scan
    g_t = r*g_{t-1} + w_t with w_t = e^{-i theta t} Bu_t, one DVE
    tensor_tensor_scan per 512-wide l-tile, chained via its initial
    value.  Pre-rotation m1/m2 on DVE (bf16 out), cross-partition
    half-combine on PE via a +-(1/64) identity stack, post-rotation
    G = [cos;sin]*g on POOL, readout contracts G against [C^T;-C^T].
  * Residual + LN: x = y2 + u kept in bf16 (its quantization noise is
    ~0.4% rms, negligible after the d=1024 variance average), split 2
    subtiles on DVE / 2 on POOL with the row-sum accumulated on the
    fly; sum(x^2) via ACT Square(bf16 in) accumulate into a dead
    scratch; LN scalar ops on POOL/DVE, Sqrt on ACT; normalize as DVE
    tensor_scalar (bf16 in/out, 2x DVE rate).
  * Four-stage software pipeline: S0 DMA issue (uth8/tg/un for tile
    it), S1 Bu matmul (it-1), S2 rotation+scan (it-2), S3 readout+
    residual+LN+store (it-3), so loads run 2-3 tiles ahead of use and
    each engine's in-order queue sees next-tile scan work before the
    current tile's LN tail.
"""

import numpy as np

import concourse.mybir as mybir
import concourse.tile as tile
from concourse import bacc, bass_utils
from concourse.bass import MemorySpace
from concourse.mybir import ActivationFunctionType as act
from concourse.mybir import AluOpType as alu

F32 = mybir.dt.float32
BF16 = mybir.dt.bfloat16
FP8 = mybir.dt.float8e4
P = 128          # partitions
L = 4096         # sequence length per core
DM = 1024        # d_model
NS = 64          # d_state
LT = 512         # l-tile (scan slice, matmul moving width)
NSUB = LT // P   # 4 l-subtiles of 128 rows per l-tile
NT = L // LT     # 8 l-tiles
KC = DM // P     # 8 contraction chunks of 128
NCORES = 8
LN_EPS = 1e-5
DH = 512         # d-model half (psum bank width)
BB_SCALE = 64.0  # fp8 b_bar pre-scale, compensated in the half-mix stack


def _build_program(use_gb: bool):
    """Builds the single-core Bass/Tile program (SPMD across 8 cores)."""
    nc = bacc.Bacc("TRN2", num_devices=NCORES, debug=False)

    # host-pre-tiled layouts: every per-l-tile DMA slice is fully contiguous
    # per partition
    un_d = nc.dram_tensor("un", [P, NT, NSUB, DM], BF16, kind="ExternalInput").ap()
    uth_d = nc.dram_tensor("uth", [P, NT, KC, LT], FP8, kind="ExternalInput").ap()
    tg_d = nc.dram_tensor("tg", [P, NT, 2, LT], BF16, kind="ExternalInput").ap()
    bb_d = nc.dram_tensor("bb", [P, KC, P], FP8, kind="ExternalInput").ap()
    cs_d = nc.dram_tensor("cs", [P, DM + P], BF16, kind="ExternalInput").ap()
    rt_d = nc.dram_tensor("rt", [P, LT], F32, kind="ExternalInput").ap()
    if use_gb:
        gam_d = nc.dram_tensor("gam", [P, DM], F32, kind="ExternalInput").ap()
        bet_d = nc.dram_tensor("bet", [P, DM], F32, kind="ExternalInput").ap()
    out_d = nc.dram_tensor("out", [P, NT, NSUB, DM], BF16, kind="ExternalOutput").ap()

    with tile.TileContext(nc) as tc:
        with (
            tc.tile_pool(name="singles", bufs=1) as singles,
            tc.tile_pool(name="un", bufs=5) as un_pool,
            tc.tile_pool(name="ut", bufs=3) as ut_pool,
            tc.tile_pool(name="tg", bufs=5) as tg_pool,
            tc.tile_pool(name="m", bufs=6) as m_pool,
            tc.tile_pool(name="g", bufs=3) as g_pool,
            tc.tile_pool(name="gg", bufs=4) as gg_pool,
            tc.tile_pool(name="x", bufs=8) as x_pool,
            tc.tile_pool(name="sqs", bufs=1) as sqs_pool,
            tc.tile_pool(name="o", bufs=3) as o_pool,
            tc.tile_pool(name="st", bufs=3) as st_pool,
            tc.tile_pool(name="pb", bufs=1, space=MemorySpace.PSUM) as psum_b,
            tc.tile_pool(name="pw", bufs=2, space=MemorySpace.PSUM) as psum_w,
            tc.tile_pool(name="py", bufs=2, space=MemorySpace.PSUM) as psum_y,
        ):
            bb_s = singles.tile([P, KC, P], FP8)
            nc.sync.dma_start(bb_s[:], bb_d)
            # cs/rt DMAs are issued inside the loop after tile 0's loads so
            # the first Bu matmul's input arrives as early as possible
            cs_s = singles.tile([P, DM + P], BF16)
            ct2_s = cs_s[:, 0:DM]
            smix_s = cs_s[:, DM : DM + P]
            rt_s = singles.tile([P, LT], F32)
            eps_s = singles.tile([P, 1], F32)
            nc.gpsimd.memset(eps_s[:], LN_EPS)
            if use_gb:
                gam_s = singles.tile([P, DM], F32)
                nc.sync.dma_start(gam_s[:], gam_d)
                bet_s = singles.tile([P, DM], F32)
                nc.sync.dma_start(bet_s[:], bet_d)

            # dead stores for the Square/accumulate passes (one per engine so
            # the repeated writes never cross-engine-sync)
            sqs_a = sqs_pool.tile([P, DM], BF16)
            sqs_p = sqs_pool.tile([P, DM], BF16)

            g_prev = None
            ut_stash = {}
            tg_stash = {}
            un_stash = {}
            m_stash = {}
            gb_stash = {}

            def _s1_s2a(j1):
                # ---- S1: Bu matmul (fp8 DoubleRow), PE-last so the PE queue
                # priority per iteration is half-mix > readout > next Bu ----
                th_t = ut_stash.pop(j1)
                bu = psum_b.tile([P, LT], F32, tag="bu")
                for k in range(KC // 2):
                    nc.tensor.matmul(
                        bu[:],
                        bb_s[:, 2 * k : 2 * k + 2, :],
                        th_t[:, 2 * k : 2 * k + 2, :],
                        start=(k == 0),
                        stop=(k == KC // 2 - 1),
                        perf_mode=mybir.MatmulPerfMode.DoubleRow,
                    )
                # ---- S2a: rotations for tile `j1`, one iteration ahead of
                # the half-mix so the PE never stalls on them.  Drain Bu
                # PSUM -> SBUF bf16 on ACT; rotations all-bf16 on POOL
                # (latency-tolerant tail work: a full iteration of slack).
                tg_t = tg_stash[j1]
                bu_s = m_pool.tile([P, LT], BF16, tag="bus")
                nc.scalar.activation(bu_s[:], bu[:], act.Identity)
                # m1 = [cos;sin]*bu, m2 = [sin;cos]*bu (bf16 out so the
                # half-mix runs as cheap bf16 matmuls)
                m1 = m_pool.tile([P, LT], BF16, tag="m1")
                nc.vector.tensor_tensor(m1[:], tg_t[:, 0, :], bu_s[:], alu.mult)
                m2 = m_pool.tile([P, LT], BF16, tag="m2")
                nc.vector.tensor_tensor(m2[:], tg_t[:, 1, :], bu_s[:], alu.mult)
                m_stash[j1] = (m1, m2)

            for it in range(NT + 4):
                # ---- S0: DMA issue for tile `it` --------------------------
                if it < NT:
                    th_t = ut_pool.tile([P, KC, LT], FP8, tag="uth")
                    nc.sync.dma_start(th_t[:], uth_d[:, it])
                    ut_stash[it] = th_t
                    tg_t = tg_pool.tile([P, 2, LT], BF16, tag="tg")
                    nc.sync.dma_start(tg_t[:], tg_d[:, it])
                    tg_stash[it] = tg_t
                    if it == 0:
                        nc.sync.dma_start(cs_s[:], cs_d)
                        nc.sync.dma_start(rt_s[:], rt_d)
                    un_t = un_pool.tile([P, NSUB, DM], BF16, tag="un")
                    nc.sync.dma_start(un_t[:], un_d[:, it])
                    un_stash[it] = un_t

                # ---- S2b: half-mix + scan + post-rotation for `it-2` ------
                j2 = it - 2
                if 0 <= j2 < NT:
                    m1, m2 = m_stash.pop(j2)
                    tg_t = tg_stash[j2]
                    # half-mix on PE: w_lo = m1_lo + m1_hi, w_hi = m2_hi-m2_lo
                    # via stationary +-(1/BB_SCALE) identity stacks
                    w = psum_w.tile([P, LT], F32, tag="w")
                    nc.tensor.matmul(
                        w[0:NS, :], smix_s[:, 0:NS], m1[:], start=True, stop=True
                    )
                    nc.tensor.matmul(
                        w[NS:P, :], smix_s[:, NS:P], m2[:], start=True, stop=True
                    )
                    # damped real scan (DVE), chained across l-tiles
                    g = g_pool.tile([P, LT], F32, tag="g")
                    init = 0.0 if g_prev is None else g_prev[:, LT - 1 : LT]
                    nc.vector.tensor_tensor_scan(
                        g[:], rt_s[:], w[:], init, alu.mult, alu.add
                    )
                    g_prev = g
                    # post-rotation G = [cos;sin]*g in bf16, first in POOL's
                    # per-iteration queue; the PE absorbs
                    # h_re = cos*g_re - sin*g_im via ct2 = [C^T; -C^T]
                    gb = gg_pool.tile([P, LT], BF16, tag="gb")
                    nc.gpsimd.tensor_tensor(gb[:], tg_t[:, 0, :], g[:], alu.mult)
                    gb_stash[j2] = gb

                # ---- S3: readout + residual + LN for tile `it-4`;  the
                # 2-iteration gap from S2b means the readout never waits on
                # the previous tile's LN tail even when POOL lags ----------
                jt = it - 4
                if jt < 0:
                    if 0 <= it - 1 < NT:
                        _s1_s2a(it - 1)
                    continue
                gb = gb_stash.pop(jt)
                tg_stash.pop(jt)
                un_t = un_stash.pop(jt)

                # readout + residual + LN per 128-row l-subtile, processed as
                # two subtile PAIRS with the LN stats computed per pair so
                # normalize(0,1) never waits on square(3) — this shortens the
                # cross-engine latency chain that otherwise phase-lags POOL.
                sx = st_pool.tile([P, NSUB], F32, tag="sx")
                sq = st_pool.tile([P, NSUB], F32, tag="sq")
                o_t = o_pool.tile([P, NSUB, DM], BF16, tag="o")
                x_list = []
                for pr in range(2):
                    for ls in (2 * pr, 2 * pr + 1):
                        lhsT = gb[:, ls * P : (ls + 1) * P]
                        x = x_pool.tile([P, DM], BF16, tag="x")
                        y2 = psum_y.tile([P, DM], F32, tag="y")
                        for dh in range(2):
                            sl = slice(dh * DH, (dh + 1) * DH)
                            nc.tensor.matmul(
                                y2[:, sl], lhsT, ct2_s[:, sl], start=True, stop=True
                            )
                        nc.vector.scalar_tensor_tensor(
                            x[:],
                            y2[:],
                            1.0,
                            un_t[:, ls, :],
                            alu.mult,
                            alu.add,
                            accum_out=sx[:, ls : ls + 1],
                        )
                        # sum(x^2) on ACT (x is bf16 SBUF)
                        nc.scalar.activation(
                            sqs_a[:], x[:], act.Square, accum_out=sq[:, ls : ls + 1]
                        )
                        x_list.append(x)

                    # LN stats for this pair (tiny [P,2] ops on POOL):
                    # mu = sx/DM ; var = sq/DM - mu^2 ; sd = sqrt(var+eps) on
                    # ACT ; rstd = 1/sd on DVE ; nmr = -mu*rstd
                    c = slice(2 * pr, 2 * pr + 2)
                    mu = st_pool.tile([P, 2], F32, tag="mu")
                    nc.gpsimd.tensor_scalar_mul(mu[:], sx[:, c], 1.0 / DM)
                    ex2 = st_pool.tile([P, 2], F32, tag="ex2")
                    nc.gpsimd.tensor_scalar_mul(ex2[:], sq[:, c], 1.0 / DM)
                    var = st_pool.tile([P, 2], F32, tag="var")
                    nc.gpsimd.tensor_tensor(var[:], mu[:], mu[:], alu.mult)
                    nc.gpsimd.tensor_tensor(var[:], ex2[:], var[:], alu.subtract)
                    sd = st_pool.tile([P, 2], F32, tag="sd")
                    nc.scalar.activation(sd[:], var[:], act.Sqrt, bias=eps_s[:, 0:1])
                    rstd = st_pool.tile([P, 2], F32, tag="rstd")
                    nc.vector.reciprocal(rstd[:], sd[:])
                    nmr = st_pool.tile([P, 2], F32, tag="nmr")
                    nc.gpsimd.tensor_tensor(nmr[:], mu[:], rstd[:], alu.mult)
                    nc.gpsimd.tensor_scalar_mul(nmr[:], nmr[:], -1.0)

                    # normalize: o = x*rstd + (-mu*rstd), bf16 in/out on POOL
                    # for steady tiles; the last two tiles alternate ACT/POOL
                    # so the drain tail isn't POOL-serial
                    for i2, ls in enumerate((2 * pr, 2 * pr + 1)):
                        if jt >= NT - 2 and ls in (0, 2):
                            nc.scalar.activation(
                                o_t[:, ls, :],
                                x_list[ls][:],
                                act.Identity,
                                bias=nmr[:, i2 : i2 + 1],
                                scale=rstd[:, i2 : i2 + 1],
                            )
                        else:
                            nc.gpsimd.tensor_scalar(
                                o_t[:, ls, :],
                                x_list[ls][:],
                                rstd[:, i2 : i2 + 1],
                                nmr[:, i2 : i2 + 1],
                                alu.mult,
                                alu.add,
                            )
                        if use_gb:
                            nc.gpsimd.tensor_tensor(
                                o_t[:, ls, :], o_t[:, ls, :], gam_s[:], alu.mult
                            )
                            nc.gpsimd.tensor_tensor(
                                o_t[:, ls, :], o_t[:, ls, :], bet_s[:], alu.add
                            )
                    nc.sync.dma_start(
                        out_d[:, jt, 2 * pr : 2 * pr + 2], o_t[:, 2 * pr : 2 * pr + 2]
                    )
                if 0 <= it - 1 < NT:
                    _s1_s2a(it - 1)
    nc.compile()
    return nc


try:
    import ml_dtypes

    ml_bf16 = ml_dtypes.bfloat16
    ml_fp8 = ml_dtypes.float8_e4m3fn
except ImportError:  # pragma: no cover
    ml_bf16 = None
    ml_fp8 = None


def _host_params(log_neg_real, imag, B_mat, C_mat):
    lnr = np.asarray(log_neg_real, np.float64)
    im = np.asarray(imag, np.float64)
    a = -np.exp(lnr) + 1j * im
    a_bar = np.exp(a)
    r = np.abs(a_bar)
    b_bar = ((a_bar - 1.0) / a)[:, None] * np.asarray(B_mat, np.float64)
    b_re = np.real(b_bar).astype(np.float32)
    b_im = np.imag(b_bar).astype(np.float32)
    # packed stationary operand for the Bu matmul: [K=d, M=128(re|im)] laid out
    # in SBUF as [128 partitions, KC, 128] with k_d = c*128 + partition
    bbT = np.concatenate([b_re, b_im], axis=0).T * BB_SCALE  # (DM, 128)
    bb = np.ascontiguousarray(bbT.reshape(KC, P, P).transpose(1, 0, 2)).astype(ml_fp8)
    ct = np.asarray(C_mat, np.float32).T  # (NS, DM)
    ct2 = np.ascontiguousarray(np.concatenate([ct, -ct], axis=0))  # (128, DM)
    # half-mix stationaries: cols 0:64 -> w_lo = (m1_lo + m1_hi)/BB_SCALE,
    # cols 64:128 -> w_hi = (m2_hi - m2_lo)/BB_SCALE (out partitions 64..127)
    eye = np.eye(NS, dtype=np.float32) / BB_SCALE
    smix = np.zeros((P, P), np.float32)
    smix[0:NS, 0:NS] = eye
    smix[NS:P, 0:NS] = eye
    smix[0:NS, NS:P] = -eye
    smix[NS:P, NS:P] = eye
    cs = np.ascontiguousarray(np.concatenate([ct2, smix], axis=1)).astype(ml_bf16)
    t = np.arange(L, dtype=np.float64)
    ang = (im[:, None] * t[None, :]) % (2 * np.pi)
    cosT = np.cos(ang).astype(np.float32)
    sinT = np.sin(ang).astype(np.float32)
    trig = np.concatenate([cosT, sinT], axis=0)  # (128, L) [cos;sin]
    trigb = np.concatenate([sinT, cosT], axis=0)  # (128, L) [sin;cos]
    # per-tile stacked trig: [P, NT, 2, LT]
    tg = np.ascontiguousarray(
        np.stack(
            [trig.reshape(P, NT, LT), trigb.reshape(P, NT, LT)], axis=2
        )
    ).astype(ml_bf16)
    rfull = np.concatenate([r, r]).astype(np.float32)
    rt = np.ascontiguousarray(np.broadcast_to(rfull[:, None], (P, LT)))
    return bb, cs, tg, rt


_PROGRAM_CACHE = {}


def kernel(u, log_neg_real, imag, B_mat, C_mat, D, gamma, beta):
    _cache = _PROGRAM_CACHE
    u = np.ascontiguousarray(np.asarray(u, np.float32))
    Dv = np.asarray(D, np.float32)
    gam = np.asarray(gamma, np.float32)
    bet = np.asarray(beta, np.float32)
    use_ures = bool(np.any(Dv != 0.0))
    use_gb = bool(np.any(gam != 1.0) or np.any(bet != 0.0))

    bb, cs, tg, rt = _host_params(log_neg_real, imag, B_mat, C_mat)

    if use_gb not in _cache:
        _cache[use_gb] = _build_program(use_gb)
    nc = _cache[use_gb]

    shared = {"bb": bb, "cs": cs, "tg": tg, "rt": rt}
    if use_gb:
        shared["gam"] = np.ascontiguousarray(
            np.broadcast_to(gam[None, :], (P, DM)).astype(np.float32)
        )
        shared["bet"] = np.ascontiguousarray(
            np.broadcast_to(bet[None, :], (P, DM)).astype(np.float32)
        )
    in_maps = []
    for b in range(NCORES):
        m = dict(shared)
        ub = u[b]
        ures = ub * (1.0 + Dv)[None, :] if use_ures else ub
        # pre-tiled [P, NT, NSUB, DM]: l = it*LT + s*128 + p
        m["un"] = np.ascontiguousarray(
            ures.astype(ml_bf16).reshape(NT, NSUB, P, DM).transpose(2, 0, 1, 3)
        )
        # pre-tiled [P, NT, KC, LT]: d = c*128 + p, l = it*LT + j
        m["uth"] = np.ascontiguousarray(
            ub.T.astype(ml_fp8).reshape(KC, P, NT, LT).transpose(1, 2, 0, 3)
        )
        in_maps.append(m)

    res = bass_utils.run_bass_kernel_spmd(nc, in_maps, core_ids=list(range(NCORES)))
    # un-permute [P, NT, NSUB, DM] -> [L, DM] and upcast
    return np.stack(
        [
            r["out"].transpose(1, 2, 0, 3).reshape(L, DM).astype(np.float32)
            for r in res.results
        ],
        axis=0,
    )


# revision 38
# speedup vs baseline: 1.0476x; 1.0476x over previous
"""Trainium2 Bass kernel for the DiagonalSSMBlock problem.

Math (per batch, sharded one batch per core over 8 cores):
    a = -exp(log_neg_real) + i*imag ; a_bar = exp(a) = r * e^{i theta}
    b_bar = ((a_bar-1)/a)[:,None] * B
    Bu_t = b_bar @ u_t                         (complex, state dim 64)
    h_t = a_bar * h_{t-1} + Bu_t               (diagonal complex scan over L)
    y_t = Re(C @ h_t) + D*u_t ; out = LN(u + y) * gamma + beta

The kernel is DMA-bound, so HBM traffic is minimized (gate is rel-err
< 2e-2; this pipeline sits at ~5e-3):
  * u ships twice: fp8(e4m3) transposed [d, l] for the Bu matmul (fp8
    feeds the matmul only; 3.5% quantization there decays to ~4e-3 in
    the LN'd output) and bf16 natural [l, d] for the residual
    (pre-scaled by (1+D) on host); both pre-tiled so every per-tile DMA
    is contiguous per partition.  Output is bf16, un-permuted on host.
  * Bu matmul runs fp8 DoubleRow (2 k-subtiles per pass, 2x PE rate);
    b_bar ships fp8 scaled by 64 and the 1/64 compensation is folded
    into the half-mix identity stack.
  * The complex scan is rotated into a per-lane REAL damped scan
    g_t = r*g_{t-1} + w_t with w_t = e^{-i theta t} Bu_t, one DVE
    tensor_tensor_scan per 512-wide l-tile, chained via its initial
    value.  Pre-rotation m1/m2 on DVE (bf16 out), cross-partition
    half-combine on PE via a +-(1/64) identity stack, post-rotation
    G = [cos;sin]*g on POOL, readout contracts G against [C^T;-C^T].
  * Residual + LN: x = y2 + u kept in bf16 (its quantization noise is
    ~0.4% rms, negligible after the d=1024 variance average), split 2
    subtiles on DVE / 2 on POOL with the row-sum accumulated on the
    fly; sum(x^2) via ACT Square(bf16 in) accumulate into a dead
    scratch; LN scalar ops on POOL/DVE, Sqrt on ACT; normalize as DVE
    tensor_scalar (bf16 in/out, 2x DVE rate).
  * Four-stage software pipeline: S0 DMA issue (uth8/tg/un for tile
    it), S1 Bu matmul (it-1), S2 rotation+scan (it-2), S3 readout+
    residual+LN+store (it-3), so loads run 2-3 tiles ahead of use and
    each engine's in-order queue sees next-tile scan work before the
    current tile's LN tail.
"""

import numpy as np

import concourse.mybir as mybir
import concourse.tile as tile
from concourse import bacc, bass_utils
from concourse.bass import MemorySpace
from concourse.mybir import ActivationFunctionType as act
from concourse.mybir import AluOpType as alu

F32 = mybir.dt.float32
BF16 = mybir.dt.bfloat16
FP8 = mybir.dt.float8e4
P = 128          # partitions
L = 4096         # sequence length per core
DM = 1024        # d_model
NS = 64          # d_state
LT = 512         # l-tile (scan slice, matmul moving width)
NSUB = LT // P   # 4 l-subtiles of 128 rows per l-tile
NT = L // LT     # 8 l-tiles
KC = DM // P     # 8 contraction chunks of 128
NCORES = 8
LN_EPS = 1e-5
DH = 512         # d-model half (psum bank width)
BB_SCALE = 64.0  # fp8 b_bar pre-scale, compensated in the half-mix stack


def _build_program(use_gb: bool):
    """Builds the single-core Bass/Tile program (SPMD across 8 cores)."""
    nc = bacc.Bacc("TRN2", num_devices=NCORES, debug=False)

    # host-pre-tiled layouts: every per-l-tile DMA slice is fully contiguous
    # per partition
    un_d = nc.dram_tensor("un", [P, NT, NSUB, DM], BF16, kind="ExternalInput").ap()
    uth_d = nc.dram_tensor("uth", [P, NT, KC, LT], FP8, kind="ExternalInput").ap()
    tg_d = nc.dram_tensor("tg", [P, NT, 2, LT], BF16, kind="ExternalInput").ap()
    bb_d = nc.dram_tensor("bb", [P, KC, P], FP8, kind="ExternalInput").ap()
    cs_d = nc.dram_tensor("cs", [P, DM + P], BF16, kind="ExternalInput").ap()
    rt_d = nc.dram_tensor("rt", [P, LT], F32, kind="ExternalInput").ap()
    if use_gb:
        gam_d = nc.dram_tensor("gam", [P, DM], F32, kind="ExternalInput").ap()
        bet_d = nc.dram_tensor("bet", [P, DM], F32, kind="ExternalInput").ap()
    out_d = nc.dram_tensor("out", [P, NT, NSUB, DM], BF16, kind="ExternalOutput").ap()

    with tile.TileContext(nc) as tc:
        with (
            tc.tile_pool(name="singles", bufs=1) as singles,
            tc.tile_pool(name="un", bufs=5) as un_pool,
            tc.tile_pool(name="ut", bufs=3) as ut_pool,
            tc.tile_pool(name="tg", bufs=5) as tg_pool,
            tc.tile_pool(name="m", bufs=6) as m_pool,
            tc.tile_pool(name="g", bufs=3) as g_pool,
            tc.tile_pool(name="gg", bufs=4) as gg_pool,
            tc.tile_pool(name="x", bufs=8) as x_pool,
            tc.tile_pool(name="sqs", bufs=1) as sqs_pool,
            tc.tile_pool(name="o", bufs=3) as o_pool,
            tc.tile_pool(name="st", bufs=3) as st_pool,
            tc.tile_pool(name="pb", bufs=1, space=MemorySpace.PSUM) as psum_b,
            tc.tile_pool(name="pw", bufs=2, space=MemorySpace.PSUM) as psum_w,
            tc.tile_pool(name="py", bufs=2, space=MemorySpace.PSUM) as psum_y,
        ):
            bb_s = singles.tile([P, KC, P], FP8)
            nc.sync.dma_start(bb_s[:], bb_d)
            # cs/rt DMAs are issued inside the loop after tile 0's loads so
            # the first Bu matmul's input arrives as early as possible
            cs_s = singles.tile([P, DM + P], BF16)
            ct2_s = cs_s[:, 0:DM]
            smix_s = cs_s[:, DM : DM + P]
            rt_s = singles.tile([P, LT], F32)
            eps_s = singles.tile([P, 1], F32)
            nc.gpsimd.memset(eps_s[:], LN_EPS)
            if use_gb:
                gam_s = singles.tile([P, DM], F32)
                nc.sync.dma_start(gam_s[:], gam_d)
                bet_s = singles.tile([P, DM], F32)
                nc.sync.dma_start(bet_s[:], bet_d)

            # dead stores for the Square/accumulate passes (one per engine so
            # the repeated writes never cross-engine-sync)
            sqs_a = sqs_pool.tile([P, DM], BF16)
            sqs_p = sqs_pool.tile([P, DM], BF16)

            g_prev = None
            ut_stash = {}
            tg_stash = {}
            un_stash = {}
            m_stash = {}
            gb_stash = {}

            def _s1_s2a(j1):
                # ---- S1: Bu matmul (fp8 DoubleRow), PE-last so the PE queue
                # priority per iteration is half-mix > readout > next Bu ----
                th_t = ut_stash.pop(j1)
                bu = psum_b.tile([P, LT], F32, tag="bu")
                for k in range(KC // 2):
                    nc.tensor.matmul(
                        bu[:],
                        bb_s[:, 2 * k : 2 * k + 2, :],
                        th_t[:, 2 * k : 2 * k + 2, :],
                        start=(k == 0),
                        stop=(k == KC // 2 - 1),
                        perf_mode=mybir.MatmulPerfMode.DoubleRow,
                    )
                # ---- S2a: rotations for tile `j1`, one iteration ahead of
                # the half-mix so the PE never stalls on them.  Drain Bu
                # PSUM -> SBUF bf16 on ACT; rotations all-bf16 on POOL
                # (latency-tolerant tail work: a full iteration of slack).
                tg_t = tg_stash[j1]
                bu_s = m_pool.tile([P, LT], BF16, tag="bus")
                nc.scalar.activation(bu_s[:], bu[:], act.Identity)
                # m1 = [cos;sin]*bu, m2 = [sin;cos]*bu (bf16 out so the
                # half-mix runs as cheap bf16 matmuls)
                m1 = m_pool.tile([P, LT], BF16, tag="m1")
                nc.vector.tensor_tensor(m1[:], tg_t[:, 0, :], bu_s[:], alu.mult)
                m2 = m_pool.tile([P, LT], BF16, tag="m2")
                nc.vector.tensor_tensor(m2[:], tg_t[:, 1, :], bu_s[:], alu.mult)
                m_stash[j1] = (m1, m2)

            for it in range(NT + 4):
                # ---- S0: DMA issue for tile `it` --------------------------
                if it < NT:
                    th_t = ut_pool.tile([P, KC, LT], FP8, tag="uth")
                    nc.sync.dma_start(th_t[:], uth_d[:, it])
                    ut_stash[it] = th_t
                    tg_t = tg_pool.tile([P, 2, LT], BF16, tag="tg")
                    nc.sync.dma_start(tg_t[:], tg_d[:, it])
                    tg_stash[it] = tg_t
                    if it == 0:
                        nc.sync.dma_start(cs_s[:], cs_d)
                        nc.sync.dma_start(rt_s[:], rt_d)
                    un_t = un_pool.tile([P, NSUB, DM], BF16, tag="un")
                    nc.sync.dma_start(un_t[:], un_d[:, it])
                    un_stash[it] = un_t

                # ---- S2b: half-mix + scan + post-rotation for `it-2` ------
                j2 = it - 2
                if 0 <= j2 < NT:
                    m1, m2 = m_stash.pop(j2)
                    tg_t = tg_stash[j2]
                    # half-mix on PE: w_lo = m1_lo + m1_hi, w_hi = m2_hi-m2_lo
                    # via stationary +-(1/BB_SCALE) identity stacks
                    w = psum_w.tile([P, LT], F32, tag="w")
                    nc.tensor.matmul(
                        w[0:NS, :], smix_s[:, 0:NS], m1[:], start=True, stop=True
                    )
                    nc.tensor.matmul(
                        w[NS:P, :], smix_s[:, NS:P], m2[:], start=True, stop=True
                    )
                    # damped real scan (DVE), chained across l-tiles
                    g = g_pool.tile([P, LT], F32, tag="g")
                    init = 0.0 if g_prev is None else g_prev[:, LT - 1 : LT]
                    nc.vector.tensor_tensor_scan(
                        g[:], rt_s[:], w[:], init, alu.mult, alu.add
                    )
                    g_prev = g
                    # post-rotation G = [cos;sin]*g in bf16, first in POOL's
                    # per-iteration queue; the PE absorbs
                    # h_re = cos*g_re - sin*g_im via ct2 = [C^T; -C^T]
                    gb = gg_pool.tile([P, LT], BF16, tag="gb")
                    nc.gpsimd.tensor_tensor(gb[:], tg_t[:, 0, :], g[:], alu.mult)
                    gb_stash[j2] = gb

                # ---- S3: readout + residual + LN for tile `it-4`;  the
                # 2-iteration gap from S2b means the readout never waits on
                # the previous tile's LN tail even when POOL lags ----------
                jt = it - 4
                if jt < 0:
                    if 0 <= it - 1 < NT:
                        _s1_s2a(it - 1)
                    continue
                gb = gb_stash.pop(jt)
                tg_stash.pop(jt)
                un_t = un_stash.pop(jt)

                # readout + residual + LN stats per 128-row l-subtile; the
                # deep S3 stage absorbs the LN chain latency, so the stats
                # run tile-batched ([P,4]) to minimize POOL's tiny-op count.
                sx = st_pool.tile([P, NSUB], F32, tag="sx")
                sq = st_pool.tile([P, NSUB], F32, tag="sq")
                o_t = o_pool.tile([P, NSUB, DM], BF16, tag="o")
                x_list = []
                for ls in range(NSUB):
                    lhsT = gb[:, ls * P : (ls + 1) * P]
                    x = x_pool.tile([P, DM], BF16, tag="x")
                    y2 = psum_y.tile([P, DM], F32, tag="y")
                    for dh in range(2):
                        sl = slice(dh * DH, (dh + 1) * DH)
                        nc.tensor.matmul(
                            y2[:, sl], lhsT, ct2_s[:, sl], start=True, stop=True
                        )
                    nc.vector.scalar_tensor_tensor(
                        x[:],
                        y2[:],
                        1.0,
                        un_t[:, ls, :],
                        alu.mult,
                        alu.add,
                        accum_out=sx[:, ls : ls + 1],
                    )
                    # sum(x^2) on ACT (x is bf16 SBUF)
                    nc.scalar.activation(
                        sqs_a[:], x[:], act.Square, accum_out=sq[:, ls : ls + 1]
                    )
                    x_list.append(x)

                # LN stats (tiny [P,4] ops on POOL): mu = sx/DM ;
                # var = sq/DM - mu^2 ; sd = sqrt(var+eps) on ACT ;
                # rstd = 1/sd on DVE ; nmr = -mu*rstd
                mu = st_pool.tile([P, NSUB], F32, tag="mu")
                nc.gpsimd.tensor_scalar_mul(mu[:], sx[:], 1.0 / DM)
                ex2 = st_pool.tile([P, NSUB], F32, tag="ex2")
                nc.gpsimd.tensor_scalar_mul(ex2[:], sq[:], 1.0 / DM)
                var = st_pool.tile([P, NSUB], F32, tag="var")
                nc.gpsimd.tensor_tensor(var[:], mu[:], mu[:], alu.mult)
                nc.gpsimd.tensor_tensor(var[:], ex2[:], var[:], alu.subtract)
                sd = st_pool.tile([P, NSUB], F32, tag="sd")
                nc.scalar.activation(sd[:], var[:], act.Sqrt, bias=eps_s[:, 0:1])
                rstd = st_pool.tile([P, NSUB], F32, tag="rstd")
                nc.vector.reciprocal(rstd[:], sd[:])
                nmr = st_pool.tile([P, NSUB], F32, tag="nmr")
                nc.gpsimd.tensor_tensor(nmr[:], mu[:], rstd[:], alu.mult)
                nc.gpsimd.tensor_scalar_mul(nmr[:], nmr[:], -1.0)

                # normalize: o = x*rstd + (-mu*rstd), bf16 in/out on POOL for
                # steady tiles; the last two tiles alternate ACT/POOL so the
                # drain tail isn't POOL-serial; store per 2 subtiles
                for ls in range(NSUB):
                    if jt >= NT - 2 and ls in (0, 2):
                        nc.scalar.activation(
                            o_t[:, ls, :],
                            x_list[ls][:],
                            act.Identity,
                            bias=nmr[:, ls : ls + 1],
                            scale=rstd[:, ls : ls + 1],
                        )
                    else:
                        nc.gpsimd.tensor_scalar(
                            o_t[:, ls, :],
                            x_list[ls][:],
                            rstd[:, ls : ls + 1],
                            nmr[:, ls : ls + 1],
                            alu.mult,
                            alu.add,
                        )
                    if use_gb:
                        nc.gpsimd.tensor_tensor(
                            o_t[:, ls, :], o_t[:, ls, :], gam_s[:], alu.mult
                        )
                        nc.gpsimd.tensor_tensor(
                            o_t[:, ls, :], o_t[:, ls, :], bet_s[:], alu.add
                        )
                    if ls % 2 == 1:
                        nc.sync.dma_start(
                            out_d[:, jt, ls - 1 : ls + 1], o_t[:, ls - 1 : ls + 1]
                        )
                if 0 <= it - 1 < NT:
                    _s1_s2a(it - 1)
    nc.compile()
    return nc


try:
    import ml_dtypes

    ml_bf16 = ml_dtypes.bfloat16
    ml_fp8 = ml_dtypes.float8_e4m3fn
except ImportError:  # pragma: no cover
    ml_bf16 = None
    ml_fp8 = None


def _host_params(log_neg_real, imag, B_mat, C_mat):
    lnr = np.asarray(log_neg_real, np.float64)
    im = np.asarray(imag, np.float64)
    a = -np.exp(lnr) + 1j * im
    a_bar = np.exp(a)
    r = np.abs(a_bar)
    b_bar = ((a_bar - 1.0) / a)[:, None] * np.asarray(B_mat, np.float64)
    b_re = np.real(b_bar).astype(np.float32)
    b_im = np.imag(b_bar).astype(np.float32)
    # packed stationary operand for the Bu matmul: [K=d, M=128(re|im)] laid out
    # in SBUF as [128 partitions, KC, 128] with k_d = c*128 + partition
    bbT = np.concatenate([b_re, b_im], axis=0).T * BB_SCALE  # (DM, 128)
    bb = np.ascontiguousarray(bbT.reshape(KC, P, P).transpose(1, 0, 2)).astype(ml_fp8)
    ct = np.asarray(C_mat, np.float32).T  # (NS, DM)
    ct2 = np.ascontiguousarray(np.concatenate([ct, -ct], axis=0))  # (128, DM)
    # half-mix stationaries: cols 0:64 -> w_lo = (m1_lo + m1_hi)/BB_SCALE,
    # cols 64:128 -> w_hi = (m2_hi - m2_lo)/BB_SCALE (out partitions 64..127)
    eye = np.eye(NS, dtype=np.float32) / BB_SCALE
    smix = np.zeros((P, P), np.float32)
    smix[0:NS, 0:NS] = eye
    smix[NS:P, 0:NS] = eye
    smix[0:NS, NS:P] = -eye
    smix[NS:P, NS:P] = eye
    cs = np.ascontiguousarray(np.concatenate([ct2, smix], axis=1)).astype(ml_bf16)
    t = np.arange(L, dtype=np.float64)
    ang = (im[:, None] * t[None, :]) % (2 * np.pi)
    cosT = np.cos(ang).astype(np.float32)
    sinT = np.sin(ang).astype(np.float32)
    trig = np.concatenate([cosT, sinT], axis=0)  # (128, L) [cos;sin]
    trigb = np.concatenate([sinT, cosT], axis=0)  # (128, L) [sin;cos]
    # per-tile stacked trig: [P, NT, 2, LT]
    tg = np.ascontiguousarray(
        np.stack(
            [trig.reshape(P, NT, LT), trigb.reshape(P, NT, LT)], axis=2
        )
    ).astype(ml_bf16)
    rfull = np.concatenate([r, r]).astype(np.float32)
    rt = np.ascontiguousarray(np.broadcast_to(rfull[:, None], (P, LT)))
    return bb, cs, tg, rt


_PROGRAM_CACHE = {}


def kernel(u, log_neg_real, imag, B_mat, C_mat, D, gamma, beta):
    _cache = _PROGRAM_CACHE
    u = np.ascontiguousarray(np.asarray(u, np.float32))
    Dv = np.asarray(D, np.float32)
    gam = np.asarray(gamma, np.float32)
    bet = np.asarray(beta, np.float32)
    use_ures = bool(np.any(Dv != 0.0))
    use_gb = bool(np.any(gam != 1.0) or np.any(bet != 0.0))

    bb, cs, tg, rt = _host_params(log_neg_real, imag, B_mat, C_mat)

    if use_gb not in _cache:
        _cache[use_gb] = _build_program(use_gb)
    nc = _cache[use_gb]

    shared = {"bb": bb, "cs": cs, "tg": tg, "rt": rt}
    if use_gb:
        shared["gam"] = np.ascontiguousarray(
            np.broadcast_to(gam[None, :], (P, DM)).astype(np.float32)
        )
        shared["bet"] = np.ascontiguousarray(
            np.broadcast_to(bet[None, :], (P, DM)).astype(np.float32)
        )
    in_maps = []
    for b in range(NCORES):
        m = dict(shared)
        ub = u[b]
        ures = ub * (1.0 + Dv)[None, :] if use_ures else ub
        # pre-tiled [P, NT, NSUB, DM]: l = it*LT + s*128 + p
        m["un"] = np.ascontiguousarray(
            ures.astype(ml_bf16).reshape(NT, NSUB, P, DM).transpose(2, 0, 1, 3)
        )
        # pre-tiled [P, NT, KC, LT]: d = c*128 + p, l = it*LT + j
        m["uth"] = np.ascontiguousarray(
            ub.T.astype(ml_fp8).reshape(KC, P, NT, LT).transpose(1, 2, 0, 3)
        )
        in_maps.append(m)

    res = bass_utils.run_bass_kernel_spmd(nc, in_maps, core_ids=list(range(NCORES)))
    # un-permute [P, NT, NSUB, DM] -> [L, DM] and upcast
    return np.stack(
        [
            r["out"].transpose(1, 2, 0, 3).reshape(L, DM).astype(np.float32)
            for r in res.results
        ],
        axis=0,
    )
